# revision 17
# baseline (speedup 1.0000x reference)
"""Trainium2 Bass kernel for nn_Encoder_5892695130542.

6-layer transformer encoder (VITS-style) with Shaw relative-position
attention (WIN=4 -> +-4 banded score bias) and K=3 conv FFN.

Sharding: data-parallel over batch. B=8 samples -> 8 NeuronCores, one
sample per core, no collectives. Weights are replicated to every core.

Per core / per layer:
  - QKV projections as bf16 matmuls (contraction C on partitions).
  - Scores per (t-chunk, head) with K=64 matmuls; the relative-position
    band bias is built with gpsimd local_scatter (per-partition indices)
    and added into the score PSUM; softmax without max-subtraction
    (logits are small), exp on ACT with fused row-sum (accum_out).
  - p and v are transposed with XBAR dma transposes for the PV matmul;
    the banded rel-V term is extracted with local_scatter, transposed
    on the PE, and folded into the PV PSUM with one extra matmul.
  - conv FFN as shifted-slice accumulating matmuls (K taps x C chunks).
  - LayerNorm over channels via ones-vector f32r matmuls (partition-dim
    reduction) + K=1 fp32 replicate matmuls to broadcast stats.
"""

import math
import os
import sys

for _p in ("/opt/trn_rl_repo", "/opt/pypackages",
           "/root/.axon_site", "/root/.axon_site/_ro/trn_rl_repo",
           "/root/.axon_site/_ro/pypackages"):
    if os.path.isdir(_p) and _p not in sys.path:
        sys.path.append(_p)

import numpy as np
import ml_dtypes

import concourse.bass as bass
import concourse.mybir as mybir
from concourse import bacc as _bacc
from concourse.bass_utils import run_bass_kernel_spmd
from concourse.tile import TileContext

BF16 = mybir.dt.bfloat16
F32 = mybir.dt.float32
F32R = mybir.dt.float32r
I16 = mybir.dt.int16
AF = mybir.ActivationFunctionType
ALU = mybir.AluOpType

B, T, C, FC, H, L, K, WIN = 8, 512, 768, 3072, 12, 6, 3, 4
DK = C // H          # 64
SCALE = 1.0 / math.sqrt(DK)
NC_ = C // 128       # 6 channel chunks
NF = FC // 128       # 24 filter chunks
NT = T // 128        # 4 time chunks
ND = 9               # band width = 2*WIN+1
BW = 144             # per-head band window width

_CACHE = {}


def _build_nc(n_layers):
    nc = _bacc.Bacc("TRN2", target_bir_lowering=False, debug=False)

    x_d = nc.dram_tensor("x", [C, T], F32, kind="ExternalInput")
    wqkv_d = nc.dram_tensor("wqkv", [n_layers, 3, NC_, 128, C], BF16, kind="ExternalInput")
    wo_d = nc.dram_tensor("wo", [n_layers, H, DK, C], BF16, kind="ExternalInput")
    w1_d = nc.dram_tensor("w1", [n_layers, 4, NC_, 128, K * 768], BF16, kind="ExternalInput")
    w2_d = nc.dram_tensor("w2", [n_layers, NF, 128, K * C], BF16, kind="ExternalInput")
    bqkv_d = nc.dram_tensor("bqkv", [n_layers, 3, NC_, 128], F32, kind="ExternalInput")
    bo_d = nc.dram_tensor("bo", [n_layers, NC_, 128], F32, kind="ExternalInput")
    b1_d = nc.dram_tensor("b1", [n_layers, NF, 128], F32, kind="ExternalInput")
    b2_d = nc.dram_tensor("b2", [n_layers, NC_, 128], F32, kind="ExternalInput")
    lng_d = nc.dram_tensor("lng", [n_layers, 4, NC_, 128], F32, kind="ExternalInput")
    ekT_d = nc.dram_tensor("ekT", [2 * DK, 16], BF16, kind="ExternalInput")
    evT_d = nc.dram_tensor("evT", [16, DK], BF16, kind="ExternalInput")
    ident_d = nc.dram_tensor("ident", [128, 128], BF16, kind="ExternalInput")
    idxpl_d = nc.dram_tensor("idxpl", [NT, 128, H * ND], I16, kind="ExternalInput")
    idxex_d = nc.dram_tensor("idxex", [128, 1024], I16, kind="ExternalInput")
    y_d = nc.dram_tensor("y", [C, T], F32, kind="ExternalOutput")

    with TileContext(nc) as tc:
        with tc.tile_pool(name="const", bufs=1) as cpool, \
             tc.tile_pool(name="big", bufs=1) as bpool, \
             tc.tile_pool(name="wq", bufs=2) as wpool, \
             tc.tile_pool(name="wc", bufs=4) as wcpool, \
             tc.tile_pool(name="pt", bufs=4) as ptpool, \
             tc.tile_pool(name="pp", bufs=8) as ppool, \
             tc.tile_pool(name="sm", bufs=4) as spool, \
             tc.tile_pool(name="st", bufs=1) as stpool, \
             tc.tile_pool(name="ps", bufs=8, space="PSUM") as ps:

            # ---- constants ----
            ekT = cpool.tile([2 * DK, 16], BF16)
            evT = cpool.tile([16, DK], BF16)
            ident = cpool.tile([128, 128], BF16)
            idxpl = cpool.tile([128, NT, H * ND], I16)
            idxex = cpool.tile([128, 1024], I16)
            nc.sync.dma_start(ekT[:], ekT_d[:])
            nc.sync.dma_start(evT[:], evT_d[:])
            nc.sync.dma_start(ident[:], ident_d[:])
            nc.sync.dma_start(idxpl[:], idxpl_d[:].rearrange("c p i -> p c i"))
            nc.sync.dma_start(idxex[:], idxex_d[:])
            onesf = cpool.tile([128, 1], F32)
            nc.vector.memset(onesf[:], 1.0)
            onesr = cpool.tile([128, 1], F32R)
            nc.vector.tensor_copy(out=onesr[:], in_=onesf[:])
            ones1 = cpool.tile([1, 128], F32)
            nc.vector.memset(ones1[:], 1.0)
            eps1 = cpool.tile([1, 1], F32)
            nc.vector.memset(eps1[:], 1e-5)

            bqkv = cpool.tile([128, n_layers, 3, NC_], F32)
            bo_s = cpool.tile([128, n_layers, NC_], F32)
            b1_s = cpool.tile([128, n_layers, NF], F32)
            b2_s = cpool.tile([128, n_layers, NC_], F32)
            lng = cpool.tile([128, n_layers, 4, NC_], F32)
            nc.sync.dma_start(bqkv[:], bqkv_d[:].rearrange("l k s p -> p l k s"))
            nc.sync.dma_start(bo_s[:], bo_d[:].rearrange("l s p -> p l s"))
            nc.sync.dma_start(b1_s[:], b1_d[:].rearrange("l s p -> p l s"))
            nc.sync.dma_start(b2_s[:], b2_d[:].rearrange("l s p -> p l s"))
            nc.sync.dma_start(lng[:], lng_d[:].rearrange("l k s p -> p l k s"))

            # ---- state tiles (reused across layers) ----
            x_sb = bpool.tile([128, NC_, T], F32)        # residual stream
            xb = bpool.tile([128, NC_, T + 2], BF16)     # bf16 copy, conv-padded
            hb = bpool.tile([128, NF, T + 2], BF16)      # FFN hidden
            q_sb = bpool.tile([128, NC_, T], BF16)
            k_sb = bpool.tile([128, NC_, T], BF16)
            v_sb = bpool.tile([128, NC_, T], BF16)
            vT = bpool.tile([128, H, NT, DK], BF16)
            attn = bpool.tile([DK, H, T], BF16)
            bpT = bpool.tile([16, H, T], BF16)
            z_all = bpool.tile([128, NT * H], F32)
            r_all = bpool.tile([128, NT * H], F32)
            r_sb = bpool.tile([128, NC_, T], F32)

            nc.vector.memset(xb[:], 0.0)
            nc.vector.memset(hb[:], 0.0)

            nc.sync.dma_start(
                x_sb[:], x_d[:].rearrange("(c p) t -> p c t", p=128))
            for ci in range(NC_):
                nc.vector.tensor_copy(out=xb[:, ci, 1:T + 1], in_=x_sb[:, ci])

            def qkslice(t_sb, h, lo, sz):
                return t_sb[(h % 2) * DK:(h % 2) * DK + DK, h // 2, lo:lo + sz]

            def layer_norm(l, src_psums, bias_col, gcol, bcol):
                """x = LN(x + psum + bias); refresh x_sb, xb."""
                for oc in range(NC_):
                    nc.vector.scalar_tensor_tensor(
                        out=r_sb[:, oc], in0=src_psums[oc][:],
                        scalar=bias_col(oc), in1=x_sb[:, oc],
                        op0=ALU.add, op1=ALU.add)
                pm = ps.tile([128, 512], F32, tag="ps")
                pq = ps.tile([128, 512], F32, tag="ps")
                for oc in range(NC_):
                    rr = spool.tile([128, T], F32R, tag="rr")
                    nc.vector.tensor_copy(out=rr[:], in_=r_sb[:, oc])
                    sq = spool.tile([128, T], F32R, tag="sq")
                    nc.scalar.square(sq[:], r_sb[:, oc])
                    nc.tensor.matmul(pm[0:1, :], onesr[:], rr[:],
                                     start=(oc == 0), stop=(oc == NC_ - 1))
                    nc.tensor.matmul(pq[0:1, :], onesr[:], sq[:],
                                     start=(oc == 0), stop=(oc == NC_ - 1))
                mean = stpool.tile([1, T], F32, tag="st0")
                ex2 = stpool.tile([1, T], F32, tag="st1")
                nc.scalar.activation(mean[:], pm[0:1, :], AF.Copy, scale=1.0 / C)
                nc.scalar.activation(ex2[:], pq[0:1, :], AF.Copy, scale=1.0 / C)
                var = stpool.tile([1, T], F32, tag="st2")
                nc.vector.scalar_tensor_tensor(
                    out=var[:], in0=mean[:], scalar=-1.0, in1=mean[:],
                    op0=ALU.mult, op1=ALU.mult)
                nc.vector.tensor_add(var[:], var[:], ex2[:])
                std = stpool.tile([1, T], F32, tag="st3")
                nc.scalar.activation(std[:], var[:], AF.Sqrt, bias=eps1[:])
                rstd = stpool.tile([1, T], F32, tag="st4")
                nc.vector.reciprocal(rstd[:], std[:])
                nmr = stpool.tile([1, T], F32, tag="st5")
                nc.vector.scalar_tensor_tensor(
                    out=nmr[:], in0=mean[:], scalar=-1.0, in1=rstd[:],
                    op0=ALU.mult, op1=ALU.mult)
                pa = ps.tile([128, 512], F32, tag="ps")
                pb = ps.tile([128, 512], F32, tag="ps")
                nc.tensor.matmul(pa[:], ones1[:], rstd[:], start=True, stop=True)
                nc.tensor.matmul(pb[:], ones1[:], nmr[:], start=True, stop=True)
                for oc in range(NC_):
                    tmp = stpool.tile([128, T], F32, tag="lntmp")
                    nc.vector.tensor_mul(tmp[:], r_sb[:, oc], pa[:])
                    nc.vector.tensor_add(tmp[:], tmp[:], pb[:])
                    nc.vector.tensor_scalar(
                        out=x_sb[:, oc], in0=tmp[:],
                        scalar1=gcol(oc), scalar2=bcol(oc),
                        op0=ALU.mult, op1=ALU.add)
                    nc.vector.tensor_copy(out=xb[:, oc, 1:T + 1], in_=x_sb[:, oc])

            NA = 0 if os.environ.get("SKIP_ATTN") == "1" else 1
            NFF = 0 if os.environ.get("SKIP_FFN") == "1" else 1
            for l in range(n_layers):
                # ---------------- attention ----------------
                for proj, dst in ((0, q_sb), (1, k_sb), (2, v_sb))[:3 * NA]:
                    pps = [ps.tile([128, 512], F32, tag="ps", name=f"pps{i}") for i in range(NC_)]
                    for ci in range(NC_):
                        wt = wpool.tile([128, C], BF16, tag="wqkv")
                        nc.sync.dma_start(wt[:], wqkv_d[l, proj, ci])
                        for oc in range(NC_):
                            nc.tensor.matmul(pps[oc][:], wt[:, oc * 128:(oc + 1) * 128],
                                             xb[:, ci, 1:T + 1],
                                             start=(ci == 0), stop=(ci == NC_ - 1))
                    for oc in range(NC_):
                        nc.scalar.activation(dst[:, oc], pps[oc][:], AF.Identity,
                                             bias=bqkv[:, l, proj, oc:oc + 1])
                for h in range(H * NA):
                    nc.scalar.dma_start_transpose(
                        vT[:, h], qkslice(v_sb, h, 0, T))

                bands = []
                for ct in range(NT * NA):
                    u_all = ppool.tile([128, H * ND], BF16, tag="u", name=f"u{ct}")
                    for h in range(H):
                        pu = ps.tile([128, 512], F32, tag="ps")
                        nc.tensor.matmul(
                            pu[:, 0:16], qkslice(q_sb, h, 128 * ct, 128),
                            ekT[(h % 2) * DK:(h % 2) * DK + DK, :],
                            start=True, stop=True)
                        nc.scalar.activation(u_all[:, h * ND:(h + 1) * ND],
                                             pu[:, 0:ND], AF.Copy)
                    band = bpool.tile([128, H * BW], BF16, tag=f"band{ct}",
                                      name=f"band{ct}")
                    nc.gpsimd.local_scatter(band[:], u_all[:], idxpl[:, ct],
                                            128, H * BW, H * ND)
                    bands.append(band)

                for h in range(H * NA):
                    pTh = ptpool.tile([128, NT, T], BF16, tag="pT", name=f"pT{h}")
                    for ct in range(NT):
                        s_lo = max(0, 128 * ct - 4)
                        s_hi = min(T, 128 * ct + BW - 4)
                        w_lo = s_lo - (128 * ct - 4)
                        psc = ps.tile([128, 512], F32, tag="ps")
                        nc.tensor.matmul(
                            psc[:], qkslice(q_sb, h, 128 * ct, 128),
                            qkslice(k_sb, h, 0, T),
                            start=True, stop=True)
                        nc.vector.tensor_add(
                            psc[:, s_lo:s_hi], psc[:, s_lo:s_hi],
                            bands[ct][:, h * BW + w_lo:h * BW + w_lo + (s_hi - s_lo)])
                        p_sb = ppool.tile([128, T], BF16, tag="p")
                        nc.scalar.activation(p_sb[:], psc[:], AF.Exp,
                                             accum_out=z_all[:, h * NT + ct:h * NT + ct + 1])
                        nc.vector.reciprocal(r_all[:, h * NT + ct:h * NT + ct + 1],
                                             z_all[:, h * NT + ct:h * NT + ct + 1])
                        nc.vector.tensor_scalar_mul(
                            p_sb[:], p_sb[:],
                            r_all[:, h * NT + ct:h * NT + ct + 1])
                        NOEX = os.environ.get("NOEX") == "1"
                        e_lo = max(0, 128 * ct - 4)
                        e_hi = min(T, 128 * ct + 132)
                        bp = spool.tile([128, 16], BF16, tag="bp")
                        if not NOEX:
                            nc.gpsimd.local_scatter(
                                bp[:], p_sb[:, e_lo:e_hi],
                                idxex[:, 512 - 128 * ct + e_lo:
                                      512 - 128 * ct + e_hi],
                                128, 16, e_hi - e_lo)
                            pt_ps = ps.tile([128, 512], BF16, tag="ps")
                            nc.tensor.transpose(pt_ps[0:16, 0:128], bp[:], ident[:])
                            nc.scalar.copy(bpT[:, h, 128 * ct:128 * ct + 128],
                                           pt_ps[0:16, 0:128])
                        teng = nc.scalar if (ct % 2 == 0) else nc.sync
                        teng.dma_start_transpose(
                            pTh[:, :, 128 * ct:128 * ct + 128], p_sb[:])
                    po = ps.tile([128, 512], F32, tag="ps")
                    for cs in range(NT):
                        nc.tensor.matmul(po[0:DK, :], vT[:, h, cs], pTh[:, cs],
                                         start=(cs == 0), stop=False)
                    if os.environ.get("NOEX") != "1":
                        nc.tensor.matmul(po[0:DK, :], evT[:], bpT[:, h],
                                         start=False, stop=True)
                    else:
                        nc.tensor.matmul(po[0:DK, :], vT[:, h, 0], pTh[:, 0],
                                         start=False, stop=True)
                    nc.scalar.activation(attn[:, h], po[0:DK, :], AF.Copy)

                # O-projection + residual + LN1
                o_ps = [ps.tile([128, 512], F32, tag="ps", name=f"ops{i}") for i in range(NC_)]
                for h in range(H * NA):
                    wt = wpool.tile([DK, C], BF16, tag="wo")
                    nc.sync.dma_start(wt[:], wo_d[l, h])
                    for oc in range(NC_):
                        nc.tensor.matmul(o_ps[oc][:], wt[:, oc * 128:(oc + 1) * 128],
                                         attn[:, h],
                                         start=(h == 0), stop=(h == H - 1))
                layer_norm(l, o_ps,
                           lambda oc: bo_s[:, l, oc:oc + 1],
                           lambda oc: lng[:, l, 0, oc:oc + 1],
                           lambda oc: lng[:, l, 1, oc:oc + 1])

                # ---------------- conv FFN ----------------
                for fg in range(4 * NFF):
                    h_ps = [ps.tile([128, 512], F32, tag="ps", name=f"hps{i}") for i in range(NC_)]
                    for ci in range(NC_):
                        wt = wcpool.tile([128, K, 768], BF16, tag="wc")
                        nc.sync.dma_start(wt[:], w1_d[l, fg, ci].rearrange(
                            "p (k f) -> p k f", k=K))
                        for fi in range(NC_):
                            for kk in range(K):
                                nc.tensor.matmul(
                                    h_ps[fi][:], wt[:, kk, fi * 128:(fi + 1) * 128],
                                    xb[:, ci, kk:kk + T],
                                    start=(ci == 0 and kk == 0),
                                    stop=(ci == NC_ - 1 and kk == K - 1))
                    for fi in range(NC_):
                        f = fg * NC_ + fi
                        nc.scalar.activation(hb[:, f, 1:T + 1], h_ps[fi][:],
                                             AF.Relu, bias=b1_s[:, l, f:f + 1])

                y_ps = [ps.tile([128, 512], F32, tag="ps", name=f"yps{i}") for i in range(NC_)]
                for fc in range(NF * NFF):
                    wt = wcpool.tile([128, K, C], BF16, tag="wc")
                    nc.sync.dma_start(wt[:], w2_d[l, fc].rearrange(
                        "p (k f) -> p k f", k=K))
                    for oc in range(NC_):
                        for kk in range(K):
                            nc.tensor.matmul(
                                y_ps[oc][:], wt[:, kk, oc * 128:(oc + 1) * 128],
                                hb[:, fc, kk:kk + T],
                                start=(fc == 0 and kk == 0),
                                stop=(fc == NF - 1 and kk == K - 1))
                layer_norm(l, y_ps,
                           lambda oc: b2_s[:, l, oc:oc + 1],
                           lambda oc: lng[:, l, 2, oc:oc + 1],
                           lambda oc: lng[:, l, 3, oc:oc + 1])

            nc.sync.dma_start(
                y_d[:].rearrange("(c p) t -> p c t", p=128), x_sb[:])
    nc.compile()
    return nc


def _prep_weights(n_layers, inputs):
    bf = ml_dtypes.bfloat16
    Wq = np.asarray(inputs["Wq"], np.float32) * SCALE
    bq = np.asarray(inputs["bq"], np.float32) * SCALE
    Wk = np.asarray(inputs["Wk"], np.float32)
    Wv = np.asarray(inputs["Wv"], np.float32)
    Wo = np.asarray(inputs["Wo"], np.float32)
    W1 = np.asarray(inputs["W1"], np.float32)
    W2 = np.asarray(inputs["W2"], np.float32)

    wqkv = np.stack([
        np.stack([w[l].T.reshape(NC_, 128, C) for w in (Wq, Wk, Wv)])
        for l in range(n_layers)])
    wo = np.stack([Wo[l].T.reshape(H, DK, C) for l in range(n_layers)])
    w1 = np.zeros((n_layers, 4, NC_, 128, K * 768), np.float32)
    for l in range(n_layers):
        t = W1[l].transpose(1, 2, 0)           # [C, K, FC]
        t = t.reshape(NC_, 128, K, 4, 768)     # [ci, p, k, fg, f]
        w1[l] = t.transpose(3, 0, 1, 2, 4).reshape(4, NC_, 128, K * 768)
    w2 = np.zeros((n_layers, NF, 128, K * C), np.float32)
    for l in range(n_layers):
        t = W2[l].transpose(1, 2, 0)           # [FC, K, C]
        w2[l] = t.reshape(NF, 128, K * C)
    bqkv = np.stack([
        np.stack([np.asarray(b, np.float32)[l].reshape(NC_, 128)
                  for b in (bq, inputs["bk"], inputs["bv"])])
        for l in range(n_layers)])
    bo = np.asarray(inputs["bo"], np.float32)[:n_layers].reshape(n_layers, NC_, 128)
    b1 = np.asarray(inputs["b1"], np.float32)[:n_layers].reshape(n_layers, NF, 128)
    b2 = np.asarray(inputs["b2"], np.float32)[:n_layers].reshape(n_layers, NC_, 128)
    lng = np.stack([
        np.stack([np.asarray(inputs[k], np.float32)[l].reshape(NC_, 128)
                  for k in ("g1", "be1", "g2", "be2")])
        for l in range(n_layers)])

    ek = np.asarray(inputs["emb_rel_k"], np.float32)[0]   # [9, DK]
    ev = np.asarray(inputs["emb_rel_v"], np.float32)[0]
    ekT = np.zeros((2 * DK, 16), np.float32)
    ekT[:DK, :ND] = ek.T
    ekT[DK:, :ND] = ek.T
    evT = np.zeros((16, DK), np.float32)
    evT[:ND] = ev

    ident = np.eye(128, dtype=np.float32)

    idxpl = np.full((NT, 128, H * ND), -1, np.int16)
    for ct in range(NT):
        for p in range(128):
            lo = max(0, 4 - (128 * ct + p))
            hi = min(ND, T - (128 * ct + p) + 4)
            for h in range(H):
                for d in range(lo, hi):
                    idxpl[ct, p, h * ND + d] = h * BW + p + d
    idxex = np.full((128, 1024), -1, np.int16)
    for p in range(128):
        qlo = max(0, p + 508)
        qhi = min(1024, p + 508 + ND)
        for q in range(qlo, qhi):
            idxex[p, q] = q - p - 508

    return dict(
        wqkv=wqkv.astype(bf), wo=wo.astype(bf),
        w1=w1.astype(bf), w2=w2.astype(bf),
        bqkv=bqkv.astype(np.float32), bo=bo.astype(np.float32),
        b1=b1.astype(np.float32), b2=b2.astype(np.float32),
        lng=lng.astype(np.float32),
        ekT=ekT.astype(bf), evT=evT.astype(bf), ident=ident.astype(bf),
        idxpl=idxpl, idxex=idxex)


def kernel(**inputs):
    n_layers = int(os.environ.get("KERNEL_LAYERS", L))
    if n_layers not in _CACHE:
        _CACHE[n_layers] = _build_nc(n_layers)
    nc = _CACHE[n_layers]
    shared = _prep_weights(n_layers, inputs)
    x = np.asarray(inputs["x"], np.float32)
    n_cores = x.shape[0]
    in_maps = []
    for b in range(n_cores):
        m = dict(shared)
        m["x"] = np.ascontiguousarray(x[b])
        in_maps.append(m)
    res = run_bass_kernel_spmd(nc, in_maps, list(range(n_cores)))
    out = np.stack([res.results[b]["y"] for b in range(n_cores)])
    return out.astype(np.float32)
